# revision 1
# baseline (speedup 1.0000x reference)
"""Trainium2 Bass kernel for batched global mean pooling (segment mean).

Computes, for N sorted nodes with 64 features and G graphs:
    out[g, f] = mean over nodes n with batch[n] == g of node_features[n, f]
(empty graphs -> zeros), distributed over 8 NeuronCores.

Strategy (graph sharding; no collectives):
  - Core k owns graphs [128k, 128(k+1)). batch is sorted, so each graph's
    nodes are a contiguous row range of node_features.
  - Host (inside kernel(), per call) lays out each core's nodes on a
    [128, T] grid: partition p gets only the nodes of local graph p,
    zero-padded to T = max graph size.
  - Features are split into fp16 (hi, lo) pairs so the PE runs at full
    rate (fp32 matmul is 4x slower); hi + lo recovers fp32 precision
    since the products accumulate into fp32 PSUM.
  - Device: each matmul is identity128.T @ slab for a [128, 7*64] fp16
    slab, accumulating into one [128, 448] f32 PSUM bank: partition =
    local graph. After all chunks: fold the 7 column blocks, multiply by
    host-provided 1/max(count, 1), DMA the [128, 64] result out.
  - Host concatenates the 8 per-core [128, 64] outputs.

The Bass program is compiled per call with the chunk count derived from
the actual input, so any node/graph distribution is handled.
"""

import math

import numpy as np

import concourse.mybir as mybir
import concourse.tile as tile
from concourse import bacc
from concourse.bass_utils import run_bass_kernel_spmd
from concourse.masks import make_identity

NCORES = 8
P = 128  # partitions = local graphs per core
F = 64  # features
B = 7  # tiles (node-rows) per matmul: N = 7*64 = 448 <= 512 f32 PSUM bank
TB = 63  # tiles per full DMA chunk (~1.03 MB per chunk)

# set by tests to capture a profile; harness path leaves these alone
TRACE = False
LAST_RESULTS = None


def _chunks(t_cap):
    """Split t_cap tiles into DMA chunks: small 21-tile chunks at the START
    (so the first chunk lands quickly and the PE starts early instead of
    trailing the stream by the whole prefetch depth) and at the END (short PE
    tail after the final DMA); full 63-tile chunks in between."""
    out = []
    t = 0
    taper = TB if t_cap > 8 * TB else 0
    while t < t_cap:
        in_taper = t < taper or t_cap - t <= taper
        n = min(3 * B if in_taper else TB, t_cap - t)
        out.append((t, n))
        t += n
    return out


def _build(t_cap):
    nc = bacc.Bacc("TRN2", target_bir_lowering=False, debug=False, num_devices=NCORES)
    # hi and lo halves of each chunk are packed back-to-back in one tensor so
    # every chunk is a single DMA: each DMA costs a semaphore, and the Tile
    # kernel-tail resets every used semaphore serially (~0.13 us each)
    hl = nc.dram_tensor(
        "hl", [P, 2 * t_cap * F], mybir.dt.float16, kind="ExternalInput"
    ).ap()
    inv = nc.dram_tensor("inv", [P, 1], mybir.dt.float32, kind="ExternalInput").ap()
    out = nc.dram_tensor("out", [P, F], mybir.dt.float32, kind="ExternalOutput").ap()

    chunks = _chunks(t_cap)
    n_mm = 2 * (t_cap // B)
    with tile.TileContext(nc) as tc:
        with (
            tc.tile_pool(name="consts", bufs=1) as consts,
            tc.tile_pool(name="io", bufs=5) as io,
            tc.tile_pool(name="ep", bufs=1) as ep,
            tc.tile_pool(name="acc", bufs=1, space="PSUM") as accp,
        ):
            # build the identity on-device (GpSimd) so the weight preload has
            # no DMA dependency -- an identity DMA would queue behind the
            # first big chunk DMAs and stall the PE ~14 us at kernel start
            ident_sb = consts.tile([P, P], mybir.dt.float16)
            make_identity(nc, ident_sb[:])

            # load the identity into the PE array once; every matmul below
            # reuses it (ldweights=False) instead of reloading 128 columns
            # per matmul (~100 ns each, ~60 us of PE time at ~600 matmuls)
            ldw = nc.tensor.ldweights(ident_sb[:])

            psum = accp.tile([P, B * F], mybir.dt.float32)
            mm = 0
            for ci, (t0, nt) in enumerate(chunks):
                hl_t = io.tile([P, 2 * TB * F], mybir.dt.float16, tag="hl")
                # alternate the two HWDGE rings (SP / ACT engines)
                eng = nc.sync if ci % 2 == 0 else nc.scalar
                eng.dma_start(
                    hl_t[:, : 2 * nt * F], hl[:, 2 * t0 * F : 2 * (t0 + nt) * F]
                )
                for b in range(2 * (nt // B)):
                    inst = nc.tensor.matmul(
                        psum[:],
                        ident_sb[:],
                        hl_t[:, b * B * F : (b + 1) * B * F],
                        start=(mm == 0),
                        stop=(mm == n_mm - 1),
                    )
                    inst.ins.ldweights = False
                    if mm == 0:
                        tile.add_dep_helper(
                            inst.ins,
                            ldw.ins,
                            sync=False,
                            reason="identity weights preloaded once",
                        )
                    mm += 1
            assert mm == n_mm

            # emitted after the chunk loop so this tiny transfer doesn't
            # head-of-line block the first chunk on the sync ring
            inv_sb = consts.tile([P, 1], mybir.dt.float32)
            nc.sync.dma_start(inv_sb[:], inv[:])

            # fold the B column blocks in ONE reduce: view psum [P, 448] as
            # [P, f=64, b=7] (b strided by 64) and sum the innermost axis --
            # one DVE op instead of a serial copy + 6 adds on the tail path
            s = ep.tile([P, F], mybir.dt.float32)
            nc.vector.tensor_reduce(
                s[:],
                psum[:, 0 : B * F].rearrange("p (b f) -> p f b", b=B),
                axis=mybir.AxisListType.X,
                op=mybir.AluOpType.add,
            )

            res = ep.tile([P, F], mybir.dt.float32)
            nc.vector.tensor_scalar_mul(res[:], s[:], inv_sb[:])
            nc.sync.dma_start(out[:], res[:])

    nc.compile()
    # bacc materializes one Ldweights per Matmult even with ldweights=False;
    # they all reload the same identity (~100 ns of PE time each). Drop the
    # redundant ones — keep any that carry semaphore waits/updates (those
    # park sync state), including the explicit preload which waits on the
    # identity build.
    for fn in nc.m.functions:
        for blk in fn.blocks:
            keep = [
                inst
                for inst in blk.instructions
                if not (
                    isinstance(inst, mybir.InstLdweights)
                    and (
                        inst.sync_info is None
                        or (
                            len(inst.sync_info.on_wait) == 0
                            and len(inst.sync_info.on_update) == 0
                        )
                    )
                )
            ]
            if len(keep) != len(blk.instructions):
                blk.instructions = keep
    return nc


def kernel(node_features, batch, num_graphs):
    global LAST_RESULTS
    x = np.asarray(node_features, dtype=np.float32)
    b = np.asarray(batch, dtype=np.int64).ravel()
    G = int(num_graphs)
    N = x.shape[0]
    assert x.shape[1] == F, f"expected {F} features, got {x.shape[1]}"

    if not np.all(b[1:] >= b[:-1]):  # defensive: layout relies on sorted batch
        order = np.argsort(b, kind="stable")
        b = b[order]
        x = x[order]

    gpc = math.ceil(G / NCORES)  # local graphs per core
    assert gpc <= P, f"num_graphs {G} too large for {NCORES} cores x {P} partitions"

    # ids >= G (if any) are dropped, matching segment_sum(num_segments=G)
    counts = np.bincount(b, minlength=NCORES * gpc)[: NCORES * gpc].astype(np.int64)
    starts = np.zeros(NCORES * gpc + 1, dtype=np.int64)
    np.cumsum(counts, out=starts[1:])
    t_max = int(counts.max()) if N else 1
    t_cap = max(B, math.ceil(t_max / B) * B)

    x_ext = np.vstack([x, np.zeros((1, F), dtype=np.float32)])  # row N = zeros
    col = np.arange(t_cap, dtype=np.int64)
    chunk_list = _chunks(t_cap)

    in_maps = []
    for k in range(NCORES):
        g0 = k * gpc
        cg = counts[g0 : g0 + gpc]
        sg = starts[g0 : g0 + gpc]
        valid = col[None, :] < cg[:, None]  # [gpc, t_cap]
        idx = np.where(valid, sg[:, None] + col[None, :], N)
        if gpc < P:  # pad partitions when graph count is not divisible by 8
            idx = np.vstack([idx, np.full((P - gpc, t_cap), N, dtype=np.int64)])

        feats = x_ext[idx]  # [P, t_cap, F] f32
        hi16 = feats.astype(np.float16).reshape(P, t_cap * F)
        lo16 = (
            (feats - hi16.reshape(P, t_cap, F).astype(np.float32))
            .astype(np.float16)
            .reshape(P, t_cap * F)
        )
        # pack [hi-chunk | lo-chunk] back-to-back per chunk (see _build)
        hl = np.empty((P, 2 * t_cap * F), dtype=np.float16)
        for t0, nt in chunk_list:
            hl[:, 2 * t0 * F : (2 * t0 + nt) * F] = hi16[:, t0 * F : (t0 + nt) * F]
            hl[:, (2 * t0 + nt) * F : 2 * (t0 + nt) * F] = lo16[:, t0 * F : (t0 + nt) * F]

        inv = np.zeros((P, 1), dtype=np.float32)
        inv[:gpc, 0] = 1.0 / np.maximum(cg, 1)
        in_maps.append({"hl": hl, "inv": inv})

    nc = _build(t_cap)
    try:
        res = run_bass_kernel_spmd(
            nc, in_maps, core_ids=list(range(NCORES)), trace=TRACE
        )
    except Exception:
        # transient device state (e.g. a previous run left a core wedged)
        # has been observed to clear on retry
        res = run_bass_kernel_spmd(
            nc, in_maps, core_ids=list(range(NCORES)), trace=TRACE
        )
    LAST_RESULTS = res

    out = np.concatenate([res.results[k]["out"] for k in range(NCORES)], axis=0)
    return out[:G]



# revision 7
# speedup vs baseline: 1.7471x; 1.7471x over previous
"""Trainium2 Bass kernel for batched global mean pooling (segment mean).

Computes, for N sorted nodes with 64 features and G graphs:
    out[g, f] = mean over nodes n with batch[n] == g of node_features[n, f]
(empty graphs -> zeros), distributed over 8 NeuronCores.

Strategy (graph sharding; no collectives):
  - Core k owns graphs [128k, 128(k+1)). batch is sorted, so each graph's
    nodes are a contiguous row range of node_features.
  - Host (inside kernel(), per call) lays out each core's nodes on a
    [128, T] grid: partition p gets only the nodes of local graph p,
    zero-padded to T = max graph size.
  - Features are cast to fp16 on host (2 bytes/elem, half the HBM
    traffic of fp32) so the PE runs at full rate; products accumulate
    into fp32 PSUM, so only the input rounding (~2^-12 relative)
    contributes error -- the mean over ~2000 nodes keeps it ~1e-4.
  - Device: each matmul is identity128.T @ slab for a [128, 7*64] fp16
    slab, accumulating into one [128, 448] f32 PSUM bank: partition =
    local graph. After all chunks: fold the 7 column blocks, multiply by
    host-provided 1/max(count, 1), DMA the [128, 64] result out.
  - Host concatenates the 8 per-core [128, 64] outputs.

The Bass program is compiled per call with the chunk count derived from
the actual input, so any node/graph distribution is handled.
"""

import math

import numpy as np

import concourse.mybir as mybir
import concourse.tile as tile
from concourse import bacc
from concourse.bass_utils import run_bass_kernel_spmd
from concourse.masks import make_identity

NCORES = 8
P = 128  # partitions = local graphs per core
F = 64  # features
B = 7  # tiles (node-rows) per matmul: N = 7*64 = 448 <= 512 f32 PSUM bank
TB = 63  # tiles per full DMA chunk (~1.03 MB per chunk)

# set by tests to capture a profile; harness path leaves these alone
TRACE = False
LAST_RESULTS = None


def _chunks(t_cap):
    """Split t_cap tiles into DMA chunks: small 21-tile chunks at the START
    (so the first chunk lands quickly and the PE starts early instead of
    trailing the stream by the whole prefetch depth) and at the END (short PE
    tail after the final DMA); full 63-tile chunks in between."""
    out = []
    t = 0
    taper = TB if t_cap > 8 * TB else 0
    while t < t_cap:
        in_taper = t < taper or t_cap - t <= taper
        n = min(3 * B if in_taper else TB, t_cap - t)
        out.append((t, n))
        t += n
    return out


def _build(t_cap):
    nc = bacc.Bacc("TRN2", target_bir_lowering=False, debug=False, num_devices=NCORES)
    hl = nc.dram_tensor(
        "hl", [P, t_cap * F], mybir.dt.float16, kind="ExternalInput"
    ).ap()
    inv = nc.dram_tensor("inv", [P, 1], mybir.dt.float32, kind="ExternalInput").ap()
    out = nc.dram_tensor("out", [P, F], mybir.dt.float32, kind="ExternalOutput").ap()

    chunks = _chunks(t_cap)
    n_mm = t_cap // B
    with tile.TileContext(nc) as tc:
        with (
            tc.tile_pool(name="consts", bufs=1) as consts,
            tc.tile_pool(name="io", bufs=5) as io,
            tc.tile_pool(name="ep", bufs=1) as ep,
            tc.tile_pool(name="acc", bufs=1, space="PSUM") as accp,
        ):
            # build the identity on-device (GpSimd) so the weight preload has
            # no DMA dependency -- an identity DMA would queue behind the
            # first big chunk DMAs and stall the PE ~14 us at kernel start
            ident_sb = consts.tile([P, P], mybir.dt.float16)
            make_identity(nc, ident_sb[:])

            # load the identity into the PE array once; every matmul below
            # reuses it (ldweights=False) instead of reloading 128 columns
            # per matmul (~100 ns each, ~60 us of PE time at ~600 matmuls)
            ldw = nc.tensor.ldweights(ident_sb[:])

            psum = accp.tile([P, B * F], mybir.dt.float32)
            mm = 0
            for ci, (t0, nt) in enumerate(chunks):
                hl_t = io.tile([P, TB * F], mybir.dt.float16, tag="hl")
                # alternate the two HWDGE rings (SP / ACT engines)
                eng = nc.sync if ci % 2 == 0 else nc.scalar
                eng.dma_start(hl_t[:, : nt * F], hl[:, t0 * F : (t0 + nt) * F])
                for b in range(nt // B):
                    inst = nc.tensor.matmul(
                        psum[:],
                        ident_sb[:],
                        hl_t[:, b * B * F : (b + 1) * B * F],
                        start=(mm == 0),
                        stop=(mm == n_mm - 1),
                    )
                    inst.ins.ldweights = False
                    if mm == 0:
                        tile.add_dep_helper(
                            inst.ins,
                            ldw.ins,
                            sync=False,
                            reason="identity weights preloaded once",
                        )
                    mm += 1
            assert mm == n_mm

            # emitted after the chunk loop so this tiny transfer doesn't
            # head-of-line block the first chunk on the sync ring
            inv_sb = consts.tile([P, 1], mybir.dt.float32)
            nc.sync.dma_start(inv_sb[:], inv[:])

            # fold the B column blocks in ONE reduce: view psum [P, 448] as
            # [P, f=64, b=7] (b strided by 64) and sum the innermost axis --
            # one DVE op instead of a serial copy + 6 adds on the tail path
            s = ep.tile([P, F], mybir.dt.float32)
            nc.vector.tensor_reduce(
                s[:],
                psum[:, 0 : B * F].rearrange("p (b f) -> p f b", b=B),
                axis=mybir.AxisListType.X,
                op=mybir.AluOpType.add,
            )

            res = ep.tile([P, F], mybir.dt.float32)
            nc.vector.tensor_scalar_mul(res[:], s[:], inv_sb[:])
            nc.sync.dma_start(out[:], res[:])

    nc.compile()
    # bacc materializes one Ldweights per Matmult even with ldweights=False;
    # they all reload the same identity (~100 ns of PE time each). Drop the
    # redundant ones — keep any that carry semaphore waits/updates (those
    # park sync state), including the explicit preload which waits on the
    # identity build.
    for fn in nc.m.functions:
        for blk in fn.blocks:
            keep = [
                inst
                for inst in blk.instructions
                if not (
                    isinstance(inst, mybir.InstLdweights)
                    and (
                        inst.sync_info is None
                        or (
                            len(inst.sync_info.on_wait) == 0
                            and len(inst.sync_info.on_update) == 0
                        )
                    )
                )
            ]
            if len(keep) != len(blk.instructions):
                blk.instructions = keep
    return nc


def kernel(node_features, batch, num_graphs):
    global LAST_RESULTS
    x = np.asarray(node_features, dtype=np.float32)
    b = np.asarray(batch, dtype=np.int64).ravel()
    G = int(num_graphs)
    N = x.shape[0]
    assert x.shape[1] == F, f"expected {F} features, got {x.shape[1]}"

    if not np.all(b[1:] >= b[:-1]):  # defensive: layout relies on sorted batch
        order = np.argsort(b, kind="stable")
        b = b[order]
        x = x[order]

    gpc = math.ceil(G / NCORES)  # local graphs per core
    assert gpc <= P, f"num_graphs {G} too large for {NCORES} cores x {P} partitions"

    # ids >= G (if any) are dropped, matching segment_sum(num_segments=G)
    counts = np.bincount(b, minlength=NCORES * gpc)[: NCORES * gpc].astype(np.int64)
    starts = np.zeros(NCORES * gpc + 1, dtype=np.int64)
    np.cumsum(counts, out=starts[1:])
    t_max = int(counts.max()) if N else 1
    t_cap = max(B, math.ceil(t_max / B) * B)

    x_ext = np.vstack([x, np.zeros((1, F), dtype=np.float32)])  # row N = zeros
    col = np.arange(t_cap, dtype=np.int64)

    in_maps = []
    for k in range(NCORES):
        g0 = k * gpc
        cg = counts[g0 : g0 + gpc]
        sg = starts[g0 : g0 + gpc]
        valid = col[None, :] < cg[:, None]  # [gpc, t_cap]
        idx = np.where(valid, sg[:, None] + col[None, :], N)
        if gpc < P:  # pad partitions when graph count is not divisible by 8
            idx = np.vstack([idx, np.full((P - gpc, t_cap), N, dtype=np.int64)])

        feats = x_ext[idx]  # [P, t_cap, F] f32
        hl = feats.astype(np.float16).reshape(P, t_cap * F)

        inv = np.zeros((P, 1), dtype=np.float32)
        inv[:gpc, 0] = 1.0 / np.maximum(cg, 1)
        in_maps.append({"hl": hl, "inv": inv})

    nc = _build(t_cap)
    try:
        res = run_bass_kernel_spmd(
            nc, in_maps, core_ids=list(range(NCORES)), trace=TRACE
        )
    except Exception:
        # transient device state (e.g. a previous run left a core wedged)
        # has been observed to clear on retry
        res = run_bass_kernel_spmd(
            nc, in_maps, core_ids=list(range(NCORES)), trace=TRACE
        )
    LAST_RESULTS = res

    out = np.concatenate([res.results[k]["out"] for k in range(NCORES)], axis=0)
    return out[:G]



# revision 8
# speedup vs baseline: 1.7477x; 1.0003x over previous
"""Trainium2 Bass kernel for batched global mean pooling (segment mean).

Computes, for N sorted nodes with 64 features and G graphs:
    out[g, f] = mean over nodes n with batch[n] == g of node_features[n, f]
(empty graphs -> zeros), distributed over 8 NeuronCores.

Strategy (graph sharding; no collectives):
  - Core k owns graphs [128k, 128(k+1)). batch is sorted, so each graph's
    nodes are a contiguous row range of node_features.
  - Host (inside kernel(), per call) lays out each core's nodes on a
    [128, T] grid: partition p gets only the nodes of local graph p,
    zero-padded to T = max graph size.
  - Features are cast to fp16 on host (2 bytes/elem, half the HBM
    traffic of fp32) so the PE runs at full rate; products accumulate
    into fp32 PSUM, so only the input rounding (~2^-12 relative)
    contributes error -- the mean over ~2000 nodes keeps it ~1e-4.
  - Device: each matmul is identity128.T @ slab for a [128, 7*64] fp16
    slab, accumulating into one [128, 448] f32 PSUM bank: partition =
    local graph. After all chunks: fold the 7 column blocks, multiply by
    host-provided 1/max(count, 1), DMA the [128, 64] result out.
  - Host concatenates the 8 per-core [128, 64] outputs.

The Bass program is compiled per call with the chunk count derived from
the actual input, so any node/graph distribution is handled.
"""

import math

import numpy as np

import concourse.mybir as mybir
import concourse.tile as tile
from concourse import bacc
from concourse.bass_utils import run_bass_kernel_spmd
from concourse.masks import make_identity

NCORES = 8
P = 128  # partitions = local graphs per core
F = 64  # features
B = 7  # tiles (node-rows) per matmul: N = 7*64 = 448 <= 512 f32 PSUM bank
TB = 126  # tiles per full DMA chunk (~2.06 MB per chunk, 16 KB per partition)

# set by tests to capture a profile; harness path leaves these alone
TRACE = False
LAST_RESULTS = None


def _chunks(t_cap):
    """Split t_cap tiles into DMA chunks: small 21-tile chunks at the START
    (so the first chunk lands quickly and the PE starts early instead of
    trailing the stream by the whole prefetch depth) and at the END (short PE
    tail after the final DMA); full 63-tile chunks in between."""
    out = []
    t = 0
    taper = TB if t_cap > 8 * TB else 0
    while t < t_cap:
        in_taper = t < taper or t_cap - t <= taper
        n = min(3 * B if in_taper else TB, t_cap - t)
        out.append((t, n))
        t += n
    return out


def _build(t_cap):
    nc = bacc.Bacc("TRN2", target_bir_lowering=False, debug=False, num_devices=NCORES)
    hl = nc.dram_tensor(
        "hl", [P, t_cap * F], mybir.dt.float16, kind="ExternalInput"
    ).ap()
    inv = nc.dram_tensor("inv", [P, 1], mybir.dt.float32, kind="ExternalInput").ap()
    out = nc.dram_tensor("out", [P, F], mybir.dt.float32, kind="ExternalOutput").ap()

    chunks = _chunks(t_cap)
    n_mm = t_cap // B
    with tile.TileContext(nc) as tc:
        with (
            tc.tile_pool(name="consts", bufs=1) as consts,
            tc.tile_pool(name="io", bufs=5) as io,
            tc.tile_pool(name="ep", bufs=1) as ep,
            tc.tile_pool(name="acc", bufs=1, space="PSUM") as accp,
        ):
            # build the identity on-device (GpSimd) so the weight preload has
            # no DMA dependency -- an identity DMA would queue behind the
            # first big chunk DMAs and stall the PE ~14 us at kernel start
            ident_sb = consts.tile([P, P], mybir.dt.float16)
            make_identity(nc, ident_sb[:])

            # load the identity into the PE array once; every matmul below
            # reuses it (ldweights=False) instead of reloading 128 columns
            # per matmul (~100 ns each, ~60 us of PE time at ~600 matmuls)
            ldw = nc.tensor.ldweights(ident_sb[:])

            psum = accp.tile([P, B * F], mybir.dt.float32)
            mm = 0
            for ci, (t0, nt) in enumerate(chunks):
                hl_t = io.tile([P, TB * F], mybir.dt.float16, tag="hl")
                # alternate the two HWDGE rings (SP / ACT engines)
                eng = nc.sync if ci % 2 == 0 else nc.scalar
                eng.dma_start(hl_t[:, : nt * F], hl[:, t0 * F : (t0 + nt) * F])
                for b in range(nt // B):
                    inst = nc.tensor.matmul(
                        psum[:],
                        ident_sb[:],
                        hl_t[:, b * B * F : (b + 1) * B * F],
                        start=(mm == 0),
                        stop=(mm == n_mm - 1),
                    )
                    inst.ins.ldweights = False
                    if mm == 0:
                        tile.add_dep_helper(
                            inst.ins,
                            ldw.ins,
                            sync=False,
                            reason="identity weights preloaded once",
                        )
                    mm += 1
            assert mm == n_mm

            # emitted after the chunk loop so this tiny transfer doesn't
            # head-of-line block the first chunk on the sync ring
            inv_sb = consts.tile([P, 1], mybir.dt.float32)
            nc.sync.dma_start(inv_sb[:], inv[:])

            # fold the B column blocks in ONE reduce: view psum [P, 448] as
            # [P, f=64, b=7] (b strided by 64) and sum the innermost axis --
            # one DVE op instead of a serial copy + 6 adds on the tail path
            s = ep.tile([P, F], mybir.dt.float32)
            nc.vector.tensor_reduce(
                s[:],
                psum[:, 0 : B * F].rearrange("p (b f) -> p f b", b=B),
                axis=mybir.AxisListType.X,
                op=mybir.AluOpType.add,
            )

            res = ep.tile([P, F], mybir.dt.float32)
            nc.vector.tensor_scalar_mul(res[:], s[:], inv_sb[:])
            nc.sync.dma_start(out[:], res[:])

    nc.compile()
    # bacc materializes one Ldweights per Matmult even with ldweights=False;
    # they all reload the same identity (~100 ns of PE time each). Drop the
    # redundant ones — keep any that carry semaphore waits/updates (those
    # park sync state), including the explicit preload which waits on the
    # identity build.
    for fn in nc.m.functions:
        for blk in fn.blocks:
            keep = [
                inst
                for inst in blk.instructions
                if not (
                    isinstance(inst, mybir.InstLdweights)
                    and (
                        inst.sync_info is None
                        or (
                            len(inst.sync_info.on_wait) == 0
                            and len(inst.sync_info.on_update) == 0
                        )
                    )
                )
            ]
            if len(keep) != len(blk.instructions):
                blk.instructions = keep
    return nc


def kernel(node_features, batch, num_graphs):
    global LAST_RESULTS
    x = np.asarray(node_features, dtype=np.float32)
    b = np.asarray(batch, dtype=np.int64).ravel()
    G = int(num_graphs)
    N = x.shape[0]
    assert x.shape[1] == F, f"expected {F} features, got {x.shape[1]}"

    if not np.all(b[1:] >= b[:-1]):  # defensive: layout relies on sorted batch
        order = np.argsort(b, kind="stable")
        b = b[order]
        x = x[order]

    gpc = math.ceil(G / NCORES)  # local graphs per core
    assert gpc <= P, f"num_graphs {G} too large for {NCORES} cores x {P} partitions"

    # ids >= G (if any) are dropped, matching segment_sum(num_segments=G)
    counts = np.bincount(b, minlength=NCORES * gpc)[: NCORES * gpc].astype(np.int64)
    starts = np.zeros(NCORES * gpc + 1, dtype=np.int64)
    np.cumsum(counts, out=starts[1:])
    t_max = int(counts.max()) if N else 1
    t_cap = max(B, math.ceil(t_max / B) * B)

    x_ext = np.vstack([x, np.zeros((1, F), dtype=np.float32)])  # row N = zeros
    col = np.arange(t_cap, dtype=np.int64)

    in_maps = []
    for k in range(NCORES):
        g0 = k * gpc
        cg = counts[g0 : g0 + gpc]
        sg = starts[g0 : g0 + gpc]
        valid = col[None, :] < cg[:, None]  # [gpc, t_cap]
        idx = np.where(valid, sg[:, None] + col[None, :], N)
        if gpc < P:  # pad partitions when graph count is not divisible by 8
            idx = np.vstack([idx, np.full((P - gpc, t_cap), N, dtype=np.int64)])

        feats = x_ext[idx]  # [P, t_cap, F] f32
        hl = feats.astype(np.float16).reshape(P, t_cap * F)

        inv = np.zeros((P, 1), dtype=np.float32)
        inv[:gpc, 0] = 1.0 / np.maximum(cg, 1)
        in_maps.append({"hl": hl, "inv": inv})

    nc = _build(t_cap)
    try:
        res = run_bass_kernel_spmd(
            nc, in_maps, core_ids=list(range(NCORES)), trace=TRACE
        )
    except Exception:
        # transient device state (e.g. a previous run left a core wedged)
        # has been observed to clear on retry
        res = run_bass_kernel_spmd(
            nc, in_maps, core_ids=list(range(NCORES)), trace=TRACE
        )
    LAST_RESULTS = res

    out = np.concatenate([res.results[k]["out"] for k in range(NCORES)], axis=0)
    return out[:G]



# revision 13
# speedup vs baseline: 1.9033x; 1.0891x over previous
"""Trainium2 Bass kernel for batched global mean pooling (segment mean).

Computes, for N sorted nodes with 64 features and G graphs:
    out[g, f] = mean over nodes n with batch[n] == g of node_features[n, f]
(empty graphs -> zeros), distributed over 8 NeuronCores.

Strategy (graph sharding; no collectives):
  - Core k owns 128 graphs. batch is sorted, so each graph's nodes are a
    contiguous row range of node_features.
  - Features are cast to fp16 on host (2 bytes/elem, half the HBM
    traffic of fp32) so the PE runs at full rate; products accumulate
    into fp32 PSUM, so only the input rounding (~2^-12 relative)
    contributes error -- the mean over ~2000 nodes keeps it ~2e-4.
  - Main stream: partition p carries the first min(c_p, 7*M0) nodes of
    local graph p, padded to 7*M0. Each matmul is identity128.T @ slab
    for a [128, 7*64] fp16 slab accumulating into PSUM bank A.
  - Overflow stream (tail of the same DMA stream): graphs larger than
    7*M0 nodes spill their remainder into overflow slots -- slot p is a
    partition-row of PSUM bank B holding up to 7*M1 nodes of ONE graph.
    This caps per-partition padding near the MEAN graph size instead of
    the max (~5.7% less HBM traffic), which matters because all 8 cores
    together saturate chip HBM bandwidth.
  - Tail: fold each bank's 7 column blocks (DVE tensor_reduce), then
    combine on the PE: out_psum = Wm.T @ fold_A + Wo.T @ fold_B where
    Wm = diag(1/count) routes partition p to graph p and Wo scatters
    overflow slots to their graphs (both host-built, fp32, and carrying
    the mean division so no separate scale op is needed). DMA the
    [128, 64] result out; host concatenates the 8 per-core outputs.

The Bass program is compiled per call with (M0, M1) derived from the
actual input, so any node/graph distribution is handled.
"""

import math

import numpy as np

import concourse.mybir as mybir
import concourse.tile as tile
from concourse import bacc
from concourse.bass_utils import run_bass_kernel_spmd
from concourse.masks import make_identity

NCORES = 8
P = 128  # partitions = local graphs per core
F = 64  # features
B = 7  # tiles (node-rows) per matmul: N = 7*64 = 448 <= 512 f32 PSUM bank
TB = 63  # nodes per full DMA chunk (~0.52 MB per chunk, 8 KB per partition)

# set by tests to capture a profile; harness path leaves these alone
TRACE = False
LAST_RESULTS = None


def _chunks(t_cap):
    """Split t_cap nodes into DMA chunks: small 21-node chunks at the START
    (so the first chunk lands quickly and the PE starts early instead of
    trailing the stream by the whole prefetch depth) and at the END (short PE
    tail after the final DMA); full 63-node chunks in between."""
    out = []
    t = 0
    taper = TB if t_cap > 8 * TB else 0
    while t < t_cap:
        in_taper = t < taper or t_cap - t <= taper
        n = min(3 * B if in_taper else TB, t_cap - t)
        out.append((t, n))
        t += n
    return out


def _build(m0, m1):
    nc = bacc.Bacc("TRN2", target_bir_lowering=False, debug=False, num_devices=NCORES)
    t_cap = (m0 + m1) * B
    hl = nc.dram_tensor(
        "hl", [P, t_cap * F], mybir.dt.float16, kind="ExternalInput"
    ).ap()
    n_w = 2 if m1 else 1
    wm = nc.dram_tensor("wm", [P, n_w * P], mybir.dt.float32, kind="ExternalInput").ap()
    out = nc.dram_tensor("out", [P, F], mybir.dt.float32, kind="ExternalOutput").ap()

    chunks = _chunks(t_cap)
    n_mm = m0 + m1
    with tile.TileContext(nc) as tc:
        with (
            tc.tile_pool(name="consts", bufs=1) as consts,
            tc.tile_pool(name="io", bufs=5) as io,
            tc.tile_pool(name="ep", bufs=1) as ep,
            tc.tile_pool(name="acc", bufs=1, space="PSUM") as accp,
        ):
            # build the identity on-device (Pool engine) so the weight preload
            # has no DMA dependency -- an identity DMA would queue behind the
            # first big chunk DMAs and stall the PE ~14 us at kernel start
            ident_sb = consts.tile([P, P], mybir.dt.float16)
            make_identity(nc, ident_sb[:])

            # load the identity into the PE array once; every streaming matmul
            # below reuses it (ldweights=False) instead of reloading 128
            # columns per matmul (~100 ns each, ~30 us of PE time)
            ldw = nc.tensor.ldweights(ident_sb[:])

            # full-bank tiles keep each accumulation group bank-aligned
            psum_a = accp.tile([P, 512], mybir.dt.float32)
            psum_b = None
            if m1:
                psum_b = accp.tile([P, 512], mybir.dt.float32, name="psum_b")
            psum_o = accp.tile([P, F], mybir.dt.float32)
            mm = 0
            for ci, (t0, nt) in enumerate(chunks):
                hl_t = io.tile([P, TB * F], mybir.dt.float16, tag="hl")
                # alternate the two HWDGE rings (SP / ACT engines)
                eng = nc.sync if ci % 2 == 0 else nc.scalar
                eng.dma_start(hl_t[:, : nt * F], hl[:, t0 * F : (t0 + nt) * F])
                for b in range(nt // B):
                    ps = psum_a if mm < m0 else psum_b
                    first = mm == 0 or mm == m0
                    last = mm == m0 - 1 or mm == n_mm - 1
                    inst = nc.tensor.matmul(
                        ps[:, : B * F],
                        ident_sb[:],
                        hl_t[:, b * B * F : (b + 1) * B * F],
                        start=first,
                        stop=last,
                    )
                    inst.ins.ldweights = False
                    if mm == 0:
                        tile.add_dep_helper(
                            inst.ins,
                            ldw.ins,
                            sync=False,
                            reason="identity weights preloaded once",
                        )
                    mm += 1
            assert mm == n_mm

            # emitted after the chunk loop so this tiny transfer doesn't
            # head-of-line block the first chunk on the sync ring
            wm_sb = consts.tile([P, n_w * P], mybir.dt.float32)
            nc.sync.dma_start(wm_sb[:], wm[:])

            # fold the B column blocks in ONE reduce: view psum [P, 448] as
            # [P, f=64, b=7] (b strided by 64) and sum the innermost axis.
            # fold_a depends only on the main matmuls, so it overlaps the
            # overflow matmul tail.
            sm = ep.tile([P, F], mybir.dt.float32)
            nc.vector.tensor_reduce(
                sm[:],
                psum_a[:, 0 : B * F].rearrange("p (b f) -> p f b", b=B),
                axis=mybir.AxisListType.X,
                op=mybir.AluOpType.add,
            )
            so = None
            if m1:
                so = ep.tile([P, F], mybir.dt.float32)
                nc.vector.tensor_reduce(
                    so[:],
                    psum_b[:, 0 : B * F].rearrange("p (b f) -> p f b", b=B),
                    axis=mybir.AxisListType.X,
                    op=mybir.AluOpType.add,
                )

            # combine + mean-divide in one PE pass: Wm/Wo carry 1/count
            nc.tensor.matmul(
                psum_o[:], wm_sb[:, 0:P], sm[:], start=True, stop=not m1
            )
            if m1:
                nc.tensor.matmul(
                    psum_o[:], wm_sb[:, P : 2 * P], so[:], start=False, stop=True
                )
            res = ep.tile([P, F], mybir.dt.float32)
            nc.vector.tensor_scalar_mul(res[:], psum_o[:], 1.0)
            nc.sync.dma_start(out[:], res[:])

    nc.compile()
    # bacc materializes one Ldweights per Matmult even with ldweights=False;
    # the streaming matmuls all reload the same identity (~100 ns of PE time
    # each). Drop exactly those reloads -- keyed on the weight tensor being
    # the identity tile -- keeping the explicit preload (it carries the wait
    # on the identity build) and the combine matmuls' Wm/Wo loads.
    ident_name = ident_sb[:].tensor.name
    for fn in nc.m.functions:
        for blk in fn.blocks:
            keep = [
                inst
                for inst in blk.instructions
                if not (
                    isinstance(inst, mybir.InstLdweights)
                    and str(inst.ins[0].memref) == ident_name
                    and (
                        inst.sync_info is None
                        or (
                            len(inst.sync_info.on_wait) == 0
                            and len(inst.sync_info.on_update) == 0
                        )
                    )
                )
            ]
            if len(keep) != len(blk.instructions):
                blk.instructions = keep
    return nc


def _plan(counts, gpc):
    """Pick (M0, M1): per-partition main/overflow matmul counts minimizing
    stream length s.t. every core's overflow fits in 128 slots of 7*M1
    nodes. counts is the per-graph node count laid out [NCORES * gpc]."""
    t_max = int(counts.max()) if counts.size else 1
    s_max = math.ceil(t_max / B)  # matmuls to cover the largest graph
    percore = counts.reshape(NCORES, gpc)
    best = (s_max, s_max, 0)  # no-overflow fallback
    for m0 in range(1, s_max):
        ovf = np.maximum(percore - B * m0, 0)
        lo = 1
        for m1 in range(lo, s_max - m0):
            if m0 + m1 >= best[0]:
                break
            slots = np.ceil(ovf / (B * m1)).sum(axis=1).max()
            if slots <= P:
                best = (m0 + m1, m0, m1)
                break
    return best[1], best[2]


def kernel(node_features, batch, num_graphs):
    global LAST_RESULTS
    x = np.asarray(node_features, dtype=np.float32)
    b = np.asarray(batch, dtype=np.int64).ravel()
    G = int(num_graphs)
    N = x.shape[0]
    assert x.shape[1] == F, f"expected {F} features, got {x.shape[1]}"

    if not np.all(b[1:] >= b[:-1]):  # defensive: layout relies on sorted batch
        order = np.argsort(b, kind="stable")
        b = b[order]
        x = x[order]

    gpc = math.ceil(G / NCORES)  # local graphs per core
    assert gpc <= P, f"num_graphs {G} too large for {NCORES} cores x {P} partitions"

    # ids >= G (if any) are dropped, matching segment_sum(num_segments=G)
    counts = np.bincount(b, minlength=NCORES * gpc)[: NCORES * gpc].astype(np.int64)
    starts = np.zeros(NCORES * gpc + 1, dtype=np.int64)
    np.cumsum(counts, out=starts[1:])
    m0, m1 = _plan(counts, gpc)
    cap0 = B * m0  # main nodes per partition
    cap1 = B * m1  # overflow nodes per slot

    x_ext = np.vstack([x, np.zeros((1, F), dtype=np.float32)])  # row N = zeros
    col0 = np.arange(cap0, dtype=np.int64)
    col1 = np.arange(cap1, dtype=np.int64) if m1 else None

    in_maps = []
    for k in range(NCORES):
        g0 = k * gpc
        cg = counts[g0 : g0 + gpc]
        sg = starts[g0 : g0 + gpc]
        inv = np.where(cg > 0, 1.0 / np.maximum(cg, 1), 0.0).astype(np.float32)

        cmain = np.minimum(cg, cap0)
        idx = np.where(col0[None, :] < cmain[:, None], sg[:, None] + col0[None, :], N)
        if gpc < P:  # pad partitions when graph count is not divisible by 8
            idx = np.vstack([idx, np.full((P - gpc, cap0), N, dtype=np.int64)])

        n_w = 2 if m1 else 1
        w = np.zeros((P, n_w * P), dtype=np.float32)
        w[np.arange(gpc), np.arange(gpc)] = inv

        if m1:
            # assign overflow slots: consecutive 7*m1-node pieces of each
            # overflow graph's tail, packed into partition-rows of stream B
            oidx = np.full((P, cap1), N, dtype=np.int64)
            slot = 0
            for g in range(gpc):
                ovf = int(cg[g] - cap0)
                pos = int(sg[g] + cap0)
                while ovf > 0:
                    take = min(ovf, cap1)
                    assert slot < P, "overflow slots exhausted (planner bug)"
                    oidx[slot, :take] = pos + np.arange(take)
                    w[slot, P + g] = inv[g]
                    pos += take
                    ovf -= take
                    slot += 1
            idx = np.hstack([idx, oidx])

        feats = x_ext[idx]  # [P, cap0(+cap1), F] f32
        hl = feats.astype(np.float16).reshape(P, -1)
        in_maps.append({"hl": hl, "wm": w})

    nc = _build(m0, m1)
    try:
        res = run_bass_kernel_spmd(
            nc, in_maps, core_ids=list(range(NCORES)), trace=TRACE
        )
    except Exception:
        # transient device state (e.g. a previous run left a core wedged)
        # has been observed to clear on retry
        res = run_bass_kernel_spmd(
            nc, in_maps, core_ids=list(range(NCORES)), trace=TRACE
        )
    LAST_RESULTS = res

    out = np.concatenate([res.results[k]["out"] for k in range(NCORES)], axis=0)
    return out[:G]


# revision 16
# speedup vs baseline: 2.2273x; 1.1702x over previous
"""Trainium2 Bass kernel for batched global mean pooling (segment mean).

Computes, for N sorted nodes with 64 features and G graphs:
    out[g, f] = mean over nodes n with batch[n] == g of node_features[n, f]
(empty graphs -> zeros), distributed over 8 NeuronCores.

Strategy (graph sharding; no collectives):
  - Core k owns 128 graphs. batch is sorted, so each graph's nodes are a
    contiguous row range of node_features.
  - Features are cast to fp16 on host (2 bytes/elem, half the HBM
    traffic of fp32) so the PE runs at full rate; products accumulate
    into fp32 PSUM, so only the input rounding (~2^-12 relative)
    contributes error -- the mean over ~2000 nodes keeps it ~2e-4.
  - Main stream: partition p carries the first min(c_p, 7*M0) nodes of
    local graph p, padded to 7*M0. Each matmul is identity128.T @ slab
    for a [128, 7*64] fp16 slab accumulating into PSUM bank A.
  - Overflow stream (tail of the same DMA stream): graphs larger than
    7*M0 nodes spill their remainder into overflow slots -- slot p is a
    partition-row of PSUM bank B holding up to 7*M1 nodes of ONE graph.
    This caps per-partition padding near the MEAN graph size instead of
    the max (~5.7% less HBM traffic), which matters because all 8 cores
    together saturate chip HBM bandwidth.
  - Tail: fold each bank's 7 column blocks (DVE tensor_reduce), then
    combine on the PE: out_psum = Wm.T @ fold_A + Wo.T @ fold_B where
    Wm = diag(1/count) routes partition p to graph p and Wo scatters
    overflow slots to their graphs (both host-built, fp32, and carrying
    the mean division so no separate scale op is needed). DMA the
    [128, 64] result out; host concatenates the 8 per-core outputs.

The Bass program is compiled per call with (M0, M1) derived from the
actual input, so any node/graph distribution is handled.
"""

import math

import numpy as np

import concourse.mybir as mybir
import concourse.tile as tile
from concourse import bacc
from concourse.bass_utils import run_bass_kernel_spmd
from concourse.masks import make_identity

NCORES = 8
P = 128  # partitions = local graphs per core
F = 64  # features
B = 7  # tiles (node-rows) per matmul: N = 7*64 = 448 <= 512 f32 PSUM bank
TB = 63  # nodes per full DMA chunk (~0.52 MB per chunk, 8 KB per partition)

# set by tests to capture a profile; harness path leaves these alone
TRACE = False
LAST_RESULTS = None


def _chunks(t_cap):
    """Split t_cap nodes into DMA chunks: small 21-node chunks at the START
    (so the first chunk lands quickly and the PE starts early instead of
    trailing the stream by the whole prefetch depth) and at the END (short PE
    tail after the final DMA); full 63-node chunks in between."""
    out = []
    t = 0
    taper = TB if t_cap > 8 * TB else 0
    while t < t_cap:
        in_taper = t < taper or t_cap - t <= taper
        n = min(3 * B if in_taper else TB, t_cap - t)
        out.append((t, n))
        t += n
    return out


def _build(m0, m1):
    nc = bacc.Bacc("TRN2", target_bir_lowering=False, debug=False, num_devices=NCORES)
    t_cap = (m0 + m1) * B
    hl = nc.dram_tensor(
        "hl", [P, t_cap * F], mybir.dt.float16, kind="ExternalInput"
    ).ap()
    n_w = 2 if m1 else 1
    wm = nc.dram_tensor("wm", [P, n_w * P], mybir.dt.float32, kind="ExternalInput").ap()
    out = nc.dram_tensor("out", [P, F], mybir.dt.float32, kind="ExternalOutput").ap()

    chunks = _chunks(t_cap)
    n_mm = m0 + m1
    with tile.TileContext(nc) as tc:
        with (
            tc.tile_pool(name="consts", bufs=1) as consts,
            tc.tile_pool(name="io", bufs=8) as io,
            tc.tile_pool(name="ep", bufs=1) as ep,
            tc.tile_pool(name="acc", bufs=1, space="PSUM") as accp,
        ):
            # build the identity on-device (Pool engine) so the weight preload
            # has no DMA dependency -- an identity DMA would queue behind the
            # first big chunk DMAs and stall the PE ~14 us at kernel start
            ident_sb = consts.tile([P, P], mybir.dt.float16)
            make_identity(nc, ident_sb[:])

            # load the identity into the PE array once; every streaming matmul
            # below reuses it (ldweights=False) instead of reloading 128
            # columns per matmul (~100 ns each, ~30 us of PE time)
            ldw = nc.tensor.ldweights(ident_sb[:])

            # full-bank tiles keep each accumulation group bank-aligned
            psum_a = accp.tile([P, 512], mybir.dt.float32)
            psum_b = None
            if m1:
                psum_b = accp.tile([P, 512], mybir.dt.float32, name="psum_b")
            psum_o = accp.tile([P, F], mybir.dt.float32)
            mm = 0
            for ci, (t0, nt) in enumerate(chunks):
                hl_t = io.tile([P, TB * F], mybir.dt.float16, tag="hl")
                # alternate the two HWDGE rings (SP / ACT engines)
                eng = nc.sync if ci % 2 == 0 else nc.scalar
                eng.dma_start(hl_t[:, : nt * F], hl[:, t0 * F : (t0 + nt) * F])
                for b in range(nt // B):
                    ps = psum_a if mm < m0 else psum_b
                    first = mm == 0 or mm == m0
                    last = mm == m0 - 1 or mm == n_mm - 1
                    inst = nc.tensor.matmul(
                        ps[:, : B * F],
                        ident_sb[:],
                        hl_t[:, b * B * F : (b + 1) * B * F],
                        start=first,
                        stop=last,
                    )
                    inst.ins.ldweights = False
                    if mm == 0:
                        tile.add_dep_helper(
                            inst.ins,
                            ldw.ins,
                            sync=False,
                            reason="identity weights preloaded once",
                        )
                    mm += 1
            assert mm == n_mm

            # emitted after the chunk loop so this tiny transfer doesn't
            # head-of-line block the first chunk on the sync ring
            wm_sb = consts.tile([P, n_w * P], mybir.dt.float32)
            nc.sync.dma_start(wm_sb[:], wm[:])

            # fold the B column blocks in ONE reduce: view psum [P, 448] as
            # [P, f=64, b=7] (b strided by 64) and sum the innermost axis.
            # fold_a depends only on the main matmuls, so it overlaps the
            # overflow matmul tail.
            sm = ep.tile([P, F], mybir.dt.float32)
            nc.vector.tensor_reduce(
                sm[:],
                psum_a[:, 0 : B * F].rearrange("p (b f) -> p f b", b=B),
                axis=mybir.AxisListType.X,
                op=mybir.AluOpType.add,
            )
            so = None
            if m1:
                so = ep.tile([P, F], mybir.dt.float32)
                nc.vector.tensor_reduce(
                    so[:],
                    psum_b[:, 0 : B * F].rearrange("p (b f) -> p f b", b=B),
                    axis=mybir.AxisListType.X,
                    op=mybir.AluOpType.add,
                )

            # combine + mean-divide in one PE pass: Wm/Wo carry 1/count
            nc.tensor.matmul(
                psum_o[:], wm_sb[:, 0:P], sm[:], start=True, stop=not m1
            )
            if m1:
                nc.tensor.matmul(
                    psum_o[:], wm_sb[:, P : 2 * P], so[:], start=False, stop=True
                )
            res = ep.tile([P, F], mybir.dt.float32)
            nc.vector.tensor_scalar_mul(res[:], psum_o[:], 1.0)
            nc.sync.dma_start(out[:], res[:])

    nc.compile()
    # bacc materializes one Ldweights per Matmult even with ldweights=False;
    # the streaming matmuls all reload the same identity (~100 ns of PE time
    # each). Drop exactly those reloads -- keyed on the weight tensor being
    # the identity tile -- keeping the explicit preload (it carries the wait
    # on the identity build) and the combine matmuls' Wm/Wo loads.
    ident_name = ident_sb[:].tensor.name
    for fn in nc.m.functions:
        for blk in fn.blocks:
            keep = [
                inst
                for inst in blk.instructions
                if not (
                    isinstance(inst, mybir.InstLdweights)
                    and str(inst.ins[0].memref) == ident_name
                    and (
                        inst.sync_info is None
                        or (
                            len(inst.sync_info.on_wait) == 0
                            and len(inst.sync_info.on_update) == 0
                        )
                    )
                )
            ]
            if len(keep) != len(blk.instructions):
                blk.instructions = keep
    # Issue the first chunk DMAs as early as possible: hoist them from the
    # tile-context body into the `main` block, ahead of the Tile preamble
    # (const memsets + all-engine barrier). They have no waits -- their
    # target buffers are fresh -- so this is pure reordering within each
    # engine's stream. Each DMA queue's first transfer pays ~4.5 us of
    # startup latency, so firing them ~0.8 us earlier (and warming four
    # queues in parallel during the preamble) pulls the whole stream left.
    for fn in nc.m.functions:
        blocks = {b.name: b for b in fn.blocks}
        main_blk = blocks.get("main")
        build_blk = next(
            (b for b in fn.blocks if "build" in b.name and not b.name.endswith("end")),
            None,
        )
        if main_blk is None or build_blk is None:
            continue
        hoist = []
        per_engine = {}
        for inst in build_blk.instructions:
            if (
                isinstance(inst, mybir.InstDMACopy)
                and per_engine.get(inst.engine, 0) < 2
                and (inst.sync_info is None or len(inst.sync_info.on_wait) == 0)
            ):
                per_engine[inst.engine] = per_engine.get(inst.engine, 0) + 1
                hoist.append(inst)
            if len(hoist) >= 4:
                break
        if hoist:
            hoist_ids = {id(i) for i in hoist}
            build_blk.instructions = [
                i for i in build_blk.instructions if id(i) not in hoist_ids
            ]
            main_blk.instructions[1:1] = hoist
    return nc


def _plan(counts, gpc):
    """Pick (M0, M1): per-partition main/overflow matmul counts minimizing
    stream length s.t. every core's overflow fits in 128 slots of 7*M1
    nodes. counts is the per-graph node count laid out [NCORES * gpc]."""
    t_max = int(counts.max()) if counts.size else 1
    s_max = math.ceil(t_max / B)  # matmuls to cover the largest graph
    percore = counts.reshape(NCORES, gpc)
    best = (s_max, s_max, 0)  # no-overflow fallback
    for m0 in range(1, s_max):
        ovf = np.maximum(percore - B * m0, 0)
        lo = 1
        for m1 in range(lo, s_max - m0):
            if m0 + m1 >= best[0]:
                break
            slots = np.ceil(ovf / (B * m1)).sum(axis=1).max()
            if slots <= P:
                best = (m0 + m1, m0, m1)
                break
    return best[1], best[2]


def kernel(node_features, batch, num_graphs):
    global LAST_RESULTS
    x = np.asarray(node_features, dtype=np.float32)
    b = np.asarray(batch, dtype=np.int64).ravel()
    G = int(num_graphs)
    N = x.shape[0]
    assert x.shape[1] == F, f"expected {F} features, got {x.shape[1]}"

    if not np.all(b[1:] >= b[:-1]):  # defensive: layout relies on sorted batch
        order = np.argsort(b, kind="stable")
        b = b[order]
        x = x[order]

    gpc = math.ceil(G / NCORES)  # local graphs per core
    assert gpc <= P, f"num_graphs {G} too large for {NCORES} cores x {P} partitions"

    # ids >= G (if any) are dropped, matching segment_sum(num_segments=G)
    counts = np.bincount(b, minlength=NCORES * gpc)[: NCORES * gpc].astype(np.int64)
    starts = np.zeros(NCORES * gpc + 1, dtype=np.int64)
    np.cumsum(counts, out=starts[1:])
    m0, m1 = _plan(counts, gpc)
    cap0 = B * m0  # main nodes per partition
    cap1 = B * m1  # overflow nodes per slot

    x_ext = np.vstack([x, np.zeros((1, F), dtype=np.float32)])  # row N = zeros
    col0 = np.arange(cap0, dtype=np.int64)
    col1 = np.arange(cap1, dtype=np.int64) if m1 else None

    in_maps = []
    for k in range(NCORES):
        g0 = k * gpc
        cg = counts[g0 : g0 + gpc]
        sg = starts[g0 : g0 + gpc]
        inv = np.where(cg > 0, 1.0 / np.maximum(cg, 1), 0.0).astype(np.float32)

        cmain = np.minimum(cg, cap0)
        idx = np.where(col0[None, :] < cmain[:, None], sg[:, None] + col0[None, :], N)
        if gpc < P:  # pad partitions when graph count is not divisible by 8
            idx = np.vstack([idx, np.full((P - gpc, cap0), N, dtype=np.int64)])

        n_w = 2 if m1 else 1
        w = np.zeros((P, n_w * P), dtype=np.float32)
        w[np.arange(gpc), np.arange(gpc)] = inv

        if m1:
            # assign overflow slots: consecutive 7*m1-node pieces of each
            # overflow graph's tail, packed into partition-rows of stream B
            oidx = np.full((P, cap1), N, dtype=np.int64)
            slot = 0
            for g in range(gpc):
                ovf = int(cg[g] - cap0)
                pos = int(sg[g] + cap0)
                while ovf > 0:
                    take = min(ovf, cap1)
                    assert slot < P, "overflow slots exhausted (planner bug)"
                    oidx[slot, :take] = pos + np.arange(take)
                    w[slot, P + g] = inv[g]
                    pos += take
                    ovf -= take
                    slot += 1
            idx = np.hstack([idx, oidx])

        feats = x_ext[idx]  # [P, cap0(+cap1), F] f32
        hl = feats.astype(np.float16).reshape(P, -1)
        in_maps.append({"hl": hl, "wm": w})

    nc = _build(m0, m1)
    try:
        res = run_bass_kernel_spmd(
            nc, in_maps, core_ids=list(range(NCORES)), trace=TRACE
        )
    except Exception:
        # transient device state (e.g. a previous run left a core wedged)
        # has been observed to clear on retry
        res = run_bass_kernel_spmd(
            nc, in_maps, core_ids=list(range(NCORES)), trace=TRACE
        )
    LAST_RESULTS = res

    out = np.concatenate([res.results[k]["out"] for k in range(NCORES)], axis=0)
    return out[:G]
